# revision 15
# baseline (speedup 1.0000x reference)
"""GNN encoder (2-layer GCN + inner-product decoder) on 8 TRN2 NeuronCores.

Sharding: nodes (rows of feat/z) are sharded across the 8 cores.
  phase 1:  x1 = feat @ W1          row-sharded GEMM (featT pre-transposed on
                                    host), AllGather x1
  phase 2:  y1 = A @ x1             per-core rows; dma_gather x1[cols]; the
                                    segment-sum is matmuls against host-built
                                    one-hot scatter matrices S (vals folded
                                    in); computed as y1T = G.T @ S
  phase 3a: x2 = y1 @ W2            from y1T, AllGather x2
  phase 3b: z = A @ x2              same spmm scheme, produces zT directly;
                                    AllGather zT (rank-blocked)
  phase 4:  A_rec = sigmoid(z_shard @ z_full.T)  row-sharded, K=64

All matmuls in fp32: the decoder logits have std ~2300 and the near-threshold
sigmoid entries are precision-critical; bf16/tf32 anywhere pushes the output
rel-err to ~1e-2.
"""

import math
import numpy as np

import concourse.bass as bass
import concourse.bacc as bacc
import concourse.mybir as mybir
import concourse.tile as tile
from concourse import bass_utils

F32 = mybir.dt.float32
I16 = mybir.dt.int16


class Cfg:
    def __init__(self, N=20000, NC=8, D=512, H=256, O=64, TB=None):
        self.N = N          # total nodes
        self.NC = NC        # cores
        self.D = D          # input feature dim
        self.H = H          # hidden dim
        self.O = O          # output dim
        assert N % NC == 0
        self.R = N // NC    # rows per core
        self.NB = (self.R + 127) // 128   # row blocks per core
        self.RP = self.NB * 128           # padded rows per core
        self.KD = D // 128  # K tiles for GEMM1
        self.KH = H // 128  # K tiles for GEMM2
        # decoder col tile: a divisor of R so tiles never cross rank blocks
        self.CW = next(cw for cw in (512, 500, 256, 128, 64)
                       if self.R % cw == 0)
        self.TB = TB        # edge tiles per row block (set by preprocess)


def preprocess(feat, adj_rows, adj_cols, adj_vals, W1, W2, cfg):
    """Host side: shard edges by destination row, group into row blocks of
    128, pad each (core, block) to a uniform TB tiles of 128 edges, and
    build the gather-index array plus the dense one-hot scatter matrices
    S[tile] (128 edges x 128 rows, entry = edge weight)."""
    N, NC, R, NB = cfg.N, cfg.NC, cfg.R, cfg.NB
    rows = np.asarray(adj_rows).astype(np.int64)
    cols = np.asarray(adj_cols).astype(np.int64)
    vals = np.asarray(adj_vals).astype(np.float32)
    feat = np.asarray(feat, dtype=np.float32)
    W1 = np.asarray(W1, dtype=np.float32)
    W2 = np.asarray(W2, dtype=np.float32)

    core_of = rows // R
    er = rows - core_of * R           # row local to core
    blk = er // 128                   # row block within core
    lr = er - blk * 128               # row local to block

    order = np.lexsort((blk, core_of))
    core_s, blk_s, lr_s, col_s, val_s = (
        core_of[order], blk[order], lr[order], cols[order], vals[order])

    counts = np.zeros((NC, NB), dtype=np.int64)
    np.add.at(counts, (core_s, blk_s), 1)
    TB = max(int(math.ceil(counts.max() / 128.0)), 1)
    cfg.TB = TB
    ntile_edges = TB * 128

    seg_end = np.cumsum(counts.reshape(-1))
    seg_start = np.concatenate([[0], seg_end[:-1]])

    in_maps = []
    for c in range(NC):
        gidx = np.zeros((NB, ntile_edges), dtype=np.int16)
        # S tiles: [NB*TB, 128 edges, 128 rows]
        stile = np.zeros((NB * TB, 128, 128), dtype=np.float32)
        for b in range(NB):
            s, e = seg_start[c * NB + b], seg_end[c * NB + b]
            n = e - s
            gidx[b, :n] = col_s[s:e].astype(np.int16)
            ti = b * TB + np.arange(n) // 128
            ei = np.arange(n) % 128
            stile[ti, ei, lr_s[s:e]] = val_s[s:e]
        gw = gidx.reshape(NB, ntile_edges // 16, 16).transpose(2, 0, 1)
        gw = gw.reshape(16, NB * (ntile_edges // 16))
        gidx_dev = np.tile(gw, (8, 1))  # idx i -> [i%16, i//16], 8x replicated
        # device layout: [128 edge-partitions, NB*TB*128 row-cols]
        stile_dev = stile.transpose(1, 0, 2).reshape(128, NB * TB * 128).copy()

        featT = np.zeros((cfg.D, cfg.RP), dtype=np.float32)
        featT[:, :R] = feat[c * R:(c + 1) * R].T

        in_maps.append({
            "featT": featT,
            "w1": W1,
            "w2": W2,
            "gidx": gidx_dev,
            "stile": stile_dev,
        })
    return in_maps, cfg


def build_bass(cfg, enable_asserts=False):
    N, NC, D, H, O = cfg.N, cfg.NC, cfg.D, cfg.H, cfg.O
    R, NB, RP, TB, CW = cfg.R, cfg.NB, cfg.RP, cfg.TB, cfg.CW
    KD, KH = cfg.KD, cfg.KH
    IE = TB * 128            # edges per block (padded)
    IW = IE // 16            # gidx columns per block

    nc = bacc.Bacc("TRN2", target_bir_lowering=False, debug=False,
                   enable_asserts=enable_asserts, num_devices=NC)

    featTd = nc.dram_tensor("featT", [D, RP], F32, kind="ExternalInput").ap()
    w1d = nc.dram_tensor("w1", [D, H], F32, kind="ExternalInput").ap()
    w2d = nc.dram_tensor("w2", [H, O], F32, kind="ExternalInput").ap()
    gidxd = nc.dram_tensor("gidx", [128, NB * IW], I16, kind="ExternalInput").ap()
    stiled = nc.dram_tensor("stile", [128, NB * TB * 128], F32,
                            kind="ExternalInput").ap()

    out_z = nc.dram_tensor("out_z", [O, R], F32, kind="ExternalOutput").ap()
    out_a = nc.dram_tensor("out_a", [R, N], F32, kind="ExternalOutput").ap()

    rg = [list(range(NC))]
    shared = "Shared" if NC > 4 else "Local"

    with tile.TileContext(nc) as tc:
        with tc.tile_pool(name="const", bufs=1) as cpool, \
             tc.tile_pool(name="dram", bufs=1, space="DRAM") as dram:

            # ---- constants / small inputs -------------------------------
            w1_t = cpool.tile([128, KD * H], F32)
            nc.sync.dma_start(
                w1_t[:, :].rearrange("p (t h) -> p t h", h=H),
                w1d.rearrange("(t p) h -> p t h", p=128))
            w2_t = cpool.tile([128, KH * O], F32)
            nc.sync.dma_start(
                w2_t[:, :].rearrange("p (t h) -> p t h", h=O),
                w2d.rearrange("(t p) h -> p t h", p=128))
            gidx_t = cpool.tile([128, NB * IW], I16)
            nc.sync.dma_start(gidx_t[:, :], gidxd)

            x1_bounce = dram.tile([R, H], F32)
            x1_full = dram.tile([N, H], F32, addr_space=shared)

            # ---- phase 1: x1 = feat @ W1, AllGather ---------------------
            with tc.tile_pool(name="p1", bufs=3) as pool, \
                 tc.tile_pool(name="p1ps", bufs=3, space="PSUM") as ps1:
                for nt in range(NB):
                    fT = pool.tile([128, KD * 128], F32, tag="fT")
                    nc.sync.dma_start(
                        fT[:, :].rearrange("p (t n) -> p t n", n=128),
                        featTd[:, nt * 128:(nt + 1) * 128]
                        .rearrange("(t p) n -> p t n", p=128))
                    x1ps = ps1.tile([128, H], F32, tag="x1ps")
                    for kt in range(KD):
                        nc.tensor.matmul(x1ps[:, :], fT[:, kt * 128:(kt + 1) * 128],
                                         w1_t[:, kt * H:(kt + 1) * H],
                                         start=(kt == 0), stop=(kt == KD - 1))
                    x1sb = pool.tile([128, H], F32, tag="x1sb")
                    nc.vector.tensor_copy(x1sb[:, :], x1ps[:, :])
                    v = min(128, R - nt * 128)
                    nc.sync.dma_start(x1_bounce[nt * 128:nt * 128 + v, :], x1sb[:v, :])

            nc.gpsimd.collective_compute(
                "AllGather", mybir.AluOpType.bypass, replica_groups=rg,
                ins=[x1_bounce.opt()], outs=[x1_full.opt()])

            # ---- phase 2: y1T = (A @ x1).T via G.T @ S ------------------
            y1cm = tc.tile_pool(name="y1p", bufs=1)
            y1pool = y1cm.__enter__()
            # y1T layout: [128 ch, (kt, b, 128 rows)] for kt in 0..KH-1
            y1T = y1pool.tile([128, KH * NB * 128], F32)
            with tc.tile_pool(name="p2g", bufs=2) as gpool, \
                 tc.tile_pool(name="p2s", bufs=2) as spool, \
                 tc.tile_pool(name="p2ps", bufs=4, space="PSUM") as ps2:
                for b in range(NB):
                    g1 = gpool.tile([128, TB * H], F32, tag="g1")
                    nc.gpsimd.dma_gather(
                        out_ap=g1[:, :].rearrange("p (t h) -> p t h", h=H),
                        in_ap=x1_full[:, :],
                        idxs_ap=gidx_t[:, b * IW:(b + 1) * IW],
                        num_idxs=IE, num_idxs_reg=IE, elem_size=H,
                        single_packet=False)
                    st = spool.tile([128, TB * 128], F32, tag="st")
                    nc.sync.dma_start(st[:, :],
                                      stiled[:, b * IE:(b + 1) * IE])
                    for kt in range(KH):
                        yps = ps2.tile([128, 128], F32, tag="yps")
                        for t in range(TB):
                            nc.tensor.matmul(
                                yps[:, :],
                                g1[:, t * H + kt * 128: t * H + (kt + 1) * 128],
                                st[:, t * 128:(t + 1) * 128],
                                start=(t == 0), stop=(t == TB - 1))
                        nc.vector.tensor_copy(
                            y1T[:, (kt * NB + b) * 128:(kt * NB + b + 1) * 128],
                            yps[:, :])

            # ---- phase 3a: x2 = y1 @ W2, AllGather ----------------------
            x2_bounce = dram.tile([R, O], F32)
            x2_full = dram.tile([N, O], F32, addr_space=shared)
            with tc.tile_pool(name="p3", bufs=3) as pool, \
                 tc.tile_pool(name="p3ps", bufs=2, space="PSUM") as ps3:
                for nt in range(NB):
                    x2ps = ps3.tile([128, O], F32, tag="x2ps")
                    for kt in range(KH):
                        nc.tensor.matmul(
                            x2ps[:, :],
                            y1T[:, (kt * NB + nt) * 128:(kt * NB + nt + 1) * 128],
                            w2_t[:, kt * O:(kt + 1) * O],
                            start=(kt == 0), stop=(kt == KH - 1))
                    x2sb = pool.tile([128, O], F32, tag="x2sb")
                    nc.vector.tensor_copy(x2sb[:, :], x2ps[:, :])
                    v = min(128, R - nt * 128)
                    nc.sync.dma_start(x2_bounce[nt * 128:nt * 128 + v, :], x2sb[:v, :])

            nc.gpsimd.collective_compute(
                "AllGather", mybir.AluOpType.bypass, replica_groups=rg,
                ins=[x2_bounce.opt()], outs=[x2_full.opt()])
            y1cm.__exit__(None, None, None)

            # ---- phase 3b: zT = (A @ x2).T via G2.T @ S -----------------
            ztcm = tc.tile_pool(name="ztp", bufs=1)
            ztpool = ztcm.__enter__()
            zTsh = ztpool.tile([O, RP], F32)
            zTall = ztpool.tile([O, N], F32)

            zT_bounce = dram.tile([O, R], F32)
            zT_full = dram.tile([NC, O, R], F32, addr_space=shared)
            with tc.tile_pool(name="p3b", bufs=2) as gpool, \
                 tc.tile_pool(name="p3bs", bufs=2) as spool, \
                 tc.tile_pool(name="p3bps", bufs=4, space="PSUM") as psz:
                for b in range(NB):
                    g2 = gpool.tile([128, TB * O], F32, tag="g2")
                    nc.gpsimd.dma_gather(
                        out_ap=g2[:, :].rearrange("p (t h) -> p t h", h=O),
                        in_ap=x2_full[:, :],
                        idxs_ap=gidx_t[:, b * IW:(b + 1) * IW],
                        num_idxs=IE, num_idxs_reg=IE, elem_size=O,
                        single_packet=False)
                    st = spool.tile([128, TB * 128], F32, tag="st2")
                    nc.sync.dma_start(st[:, :],
                                      stiled[:, b * IE:(b + 1) * IE])
                    zps = psz.tile([O, 128], F32, tag="zps")
                    for t in range(TB):
                        nc.tensor.matmul(
                            zps[:, :], g2[:, t * O:(t + 1) * O],
                            st[:, t * 128:(t + 1) * 128],
                            start=(t == 0), stop=(t == TB - 1))
                    nc.vector.tensor_copy(zTsh[:, b * 128:(b + 1) * 128], zps[:, :])
                    v = min(128, R - b * 128)
                    nc.sync.dma_start(zT_bounce[:, b * 128:b * 128 + v],
                                      zTsh[:, b * 128:b * 128 + v])

            nc.sync.dma_start(out_z[:, :], zTsh[:, :R])
            nc.gpsimd.collective_compute(
                "AllGather", mybir.AluOpType.bypass, replica_groups=rg,
                ins=[zT_bounce.opt()], outs=[zT_full.opt()])

            # zTall [O, N]: rank r's columns r*R..(r+1)*R come from its block
            nc.sync.dma_start(
                zTall[:, :].rearrange("p (r n) -> p r n", r=NC),
                zT_full.rearrange("r p n -> p r n"))

            # ---- phase 4: A_rec = sigmoid(z_shard @ z_full.T) -----------
            NCT = N // CW                  # decoder col tiles (within ranks)
            GRP = 4                        # col tiles per staging buffer
            with tc.tile_pool(name="p4o", bufs=3) as opool, \
                 tc.tile_pool(name="p4ps", bufs=8, space="PSUM") as ps4:
                for rt in range(NB):
                    v = min(128, R - rt * 128)
                    lhsT = zTsh[:, rt * 128:(rt + 1) * 128]
                    for gc in range(0, NCT, GRP):
                        nct = min(GRP, NCT - gc)
                        ab = opool.tile([128, GRP * CW], F32, tag="ab")
                        for j in range(nct):
                            ct = gc + j
                            aps = ps4.tile([128, CW], F32, tag="aps")
                            nc.tensor.matmul(aps[:, :], lhsT,
                                             zTall[:, ct * CW:(ct + 1) * CW],
                                             start=True, stop=True)
                            nc.scalar.activation(
                                ab[:, j * CW:(j + 1) * CW], aps[:, :],
                                mybir.ActivationFunctionType.Sigmoid)
                        nc.sync.dma_start(
                            out_a[rt * 128:rt * 128 + v,
                                  gc * CW:(gc + nct) * CW],
                            ab[:v, :nct * CW])
            ztcm.__exit__(None, None, None)

    nc.compile()
    return nc


_CACHE = {}


def kernel(feat, adj_rows, adj_cols, adj_vals, W1, W2):
    cfg = Cfg()
    in_maps, cfg = preprocess(feat, adj_rows, adj_cols, adj_vals, W1, W2, cfg)
    key = ("k", cfg.N, cfg.NC, cfg.TB)
    if key not in _CACHE:
        _CACHE[key] = build_bass(cfg)
    nc = _CACHE[key]
    res = bass_utils.run_bass_kernel_spmd(
        nc, in_maps, core_ids=list(range(cfg.NC)))
    z = np.concatenate([res.results[c]["out_z"].T for c in range(cfg.NC)], axis=0)
    A = np.concatenate([res.results[c]["out_a"] for c in range(cfg.NC)], axis=0)
    return z, A


# revision 20
# speedup vs baseline: 1.1422x; 1.1422x over previous
"""GNN encoder (2-layer GCN + inner-product decoder) on 8 TRN2 NeuronCores.

Sharding: nodes (rows of feat/z) are sharded across the 8 cores.
  phase 1:  x1 = feat @ W1          row-sharded GEMM (featT pre-transposed on
                                    host), AllGather x1
  phase 2:  y1 = A @ x1             per-core rows; dma_gather x1[cols]; the
                                    segment-sum is matmuls against host-built
                                    one-hot scatter matrices S (vals folded
                                    in); computed as y1T = G.T @ S
  phase 3a: x2 = y1 @ W2            from y1T, AllGather x2
  phase 3b: z = A @ x2              same spmm scheme, produces zT directly;
                                    AllGather zT (rank-blocked)
  phase 4:  A_rec = sigmoid(z_shard @ z_full.T)  row-sharded, K=64

All matmuls in fp32: the decoder logits have std ~2300 and the near-threshold
sigmoid entries are precision-critical; bf16/tf32 anywhere pushes the output
rel-err to ~1e-2.
"""

import math
import numpy as np

import concourse.bass as bass
import concourse.bacc as bacc
import concourse.mybir as mybir
import concourse.tile as tile
from concourse import bass_utils

F32 = mybir.dt.float32
I16 = mybir.dt.int16


class Cfg:
    def __init__(self, N=20000, NC=8, D=512, H=256, O=64, TB=None):
        self.N = N          # total nodes
        self.NC = NC        # cores
        self.D = D          # input feature dim
        self.H = H          # hidden dim
        self.O = O          # output dim
        assert N % NC == 0
        self.R = N // NC    # rows per core
        self.NB = (self.R + 127) // 128   # row blocks per core
        self.RP = self.NB * 128           # padded rows per core
        self.KD = D // 128  # K tiles for GEMM1
        self.KH = H // 128  # K tiles for GEMM2
        # decoder col tile: a divisor of R so tiles never cross rank blocks
        self.CW = next(cw for cw in (512, 500, 256, 128, 64)
                       if self.R % cw == 0)
        self.TB = TB        # edge tiles per row block (set by preprocess)


def preprocess(feat, adj_rows, adj_cols, adj_vals, W1, W2, cfg):
    """Host side: shard edges by destination row, group into row blocks of
    128, pad each (core, block) to a uniform TB tiles of 128 edges, and
    build the gather-index array plus the dense one-hot scatter matrices
    S[tile] (128 edges x 128 rows, entry = edge weight)."""
    N, NC, R, NB = cfg.N, cfg.NC, cfg.R, cfg.NB
    rows = np.asarray(adj_rows).astype(np.int64)
    cols = np.asarray(adj_cols).astype(np.int64)
    vals = np.asarray(adj_vals).astype(np.float32)
    feat = np.asarray(feat, dtype=np.float32)
    W1 = np.asarray(W1, dtype=np.float32)
    W2 = np.asarray(W2, dtype=np.float32)

    core_of = rows // R
    er = rows - core_of * R           # row local to core
    blk = er // 128                   # row block within core
    lr = er - blk * 128               # row local to block

    order = np.lexsort((blk, core_of))
    core_s, blk_s, lr_s, col_s, val_s = (
        core_of[order], blk[order], lr[order], cols[order], vals[order])

    counts = np.zeros((NC, NB), dtype=np.int64)
    np.add.at(counts, (core_s, blk_s), 1)
    TB = max(int(math.ceil(counts.max() / 128.0)), 1)
    cfg.TB = TB
    ntile_edges = TB * 128

    seg_end = np.cumsum(counts.reshape(-1))
    seg_start = np.concatenate([[0], seg_end[:-1]])

    in_maps = []
    for c in range(NC):
        gidx = np.zeros((NB, ntile_edges), dtype=np.int16)
        # S tiles: [NB*TB, 128 edges, 128 rows]
        stile = np.zeros((NB * TB, 128, 128), dtype=np.float32)
        for b in range(NB):
            s, e = seg_start[c * NB + b], seg_end[c * NB + b]
            n = e - s
            gidx[b, :n] = col_s[s:e].astype(np.int16)
            ti = b * TB + np.arange(n) // 128
            ei = np.arange(n) % 128
            stile[ti, ei, lr_s[s:e]] = val_s[s:e]
        gw = gidx.reshape(NB, ntile_edges // 16, 16).transpose(2, 0, 1)
        gw = gw.reshape(16, NB * (ntile_edges // 16))
        gidx_dev = np.tile(gw, (8, 1))  # idx i -> [i%16, i//16], 8x replicated
        # device layout: [128 edge-partitions, NB*TB*128 row-cols]
        stile_dev = stile.transpose(1, 0, 2).reshape(128, NB * TB * 128).copy()

        featT = np.zeros((cfg.D, cfg.RP), dtype=np.float32)
        featT[:, :R] = feat[c * R:(c + 1) * R].T

        in_maps.append({
            "featT": featT,
            "w1": W1,
            "w2": W2,
            "gidx": gidx_dev,
            "stile": stile_dev,
        })
    return in_maps, cfg


def build_bass(cfg, enable_asserts=False):
    N, NC, D, H, O = cfg.N, cfg.NC, cfg.D, cfg.H, cfg.O
    R, NB, RP, TB, CW = cfg.R, cfg.NB, cfg.RP, cfg.TB, cfg.CW
    KD, KH = cfg.KD, cfg.KH
    IE = TB * 128            # edges per block (padded)
    IW = IE // 16            # gidx columns per block

    nc = bacc.Bacc("TRN2", target_bir_lowering=False, debug=False,
                   enable_asserts=enable_asserts, num_devices=NC)

    featTd = nc.dram_tensor("featT", [D, RP], F32, kind="ExternalInput").ap()
    w1d = nc.dram_tensor("w1", [D, H], F32, kind="ExternalInput").ap()
    w2d = nc.dram_tensor("w2", [H, O], F32, kind="ExternalInput").ap()
    gidxd = nc.dram_tensor("gidx", [128, NB * IW], I16, kind="ExternalInput").ap()
    stiled = nc.dram_tensor("stile", [128, NB * TB * 128], F32,
                            kind="ExternalInput").ap()

    out_z = nc.dram_tensor("out_z", [O, R], F32, kind="ExternalOutput").ap()
    out_a = nc.dram_tensor("out_a", [R, N], F32, kind="ExternalOutput").ap()

    rg = [list(range(NC))]
    shared = "Shared" if NC > 4 else "Local"

    with tile.TileContext(nc) as tc:
        with tc.tile_pool(name="const", bufs=1) as cpool, \
             tc.tile_pool(name="dram", bufs=1, space="DRAM") as dram:

            # ---- constants / small inputs -------------------------------
            w1_t = cpool.tile([128, KD * H], F32)
            nc.sync.dma_start(
                w1_t[:, :].rearrange("p (t h) -> p t h", h=H),
                w1d.rearrange("(t p) h -> p t h", p=128))
            w2_t = cpool.tile([128, KH * O], F32)
            nc.sync.dma_start(
                w2_t[:, :].rearrange("p (t h) -> p t h", h=O),
                w2d.rearrange("(t p) h -> p t h", p=128))
            gidx_t = cpool.tile([128, NB * IW], I16)
            nc.sync.dma_start(gidx_t[:, :], gidxd)

            x1_bounce = dram.tile([R, H], F32)
            x1_full = dram.tile([N, H], F32, addr_space=shared)

            # ---- phase 1: x1 = feat @ W1, AllGather ---------------------
            with tc.tile_pool(name="p1", bufs=3) as pool, \
                 tc.tile_pool(name="p1ps", bufs=3, space="PSUM") as ps1:
                for nt in range(NB):
                    fT = pool.tile([128, KD * 128], F32, tag="fT")
                    nc.sync.dma_start(
                        fT[:, :].rearrange("p (t n) -> p t n", n=128),
                        featTd[:, nt * 128:(nt + 1) * 128]
                        .rearrange("(t p) n -> p t n", p=128))
                    x1ps = ps1.tile([128, H], F32, tag="x1ps")
                    for kt in range(KD):
                        nc.tensor.matmul(x1ps[:, :], fT[:, kt * 128:(kt + 1) * 128],
                                         w1_t[:, kt * H:(kt + 1) * H],
                                         start=(kt == 0), stop=(kt == KD - 1))
                    x1sb = pool.tile([128, H], F32, tag="x1sb")
                    nc.vector.tensor_copy(x1sb[:, :], x1ps[:, :])
                    v = min(128, R - nt * 128)
                    nc.sync.dma_start(x1_bounce[nt * 128:nt * 128 + v, :], x1sb[:v, :])

            nc.gpsimd.collective_compute(
                "AllGather", mybir.AluOpType.bypass, replica_groups=rg,
                ins=[x1_bounce.opt()], outs=[x1_full.opt()])

            # ---- phase 2: y1T = (A @ x1).T via G.T @ S ------------------
            y1cm = tc.tile_pool(name="y1p", bufs=1)
            y1pool = y1cm.__enter__()
            # y1T layout: [128 ch, (kt, b, 128 rows)] for kt in 0..KH-1
            y1T = y1pool.tile([128, KH * NB * 128], F32)
            with tc.tile_pool(name="p2g", bufs=3) as gpool, \
                 tc.tile_pool(name="p2s", bufs=2) as spool, \
                 tc.tile_pool(name="p2ps", bufs=4, space="PSUM") as ps2:
                for b in range(NB):
                    g1 = gpool.tile([128, TB * H], F32, tag="g1")
                    nc.gpsimd.dma_gather(
                        out_ap=g1[:, :].rearrange("p (t h) -> p t h", h=H),
                        in_ap=x1_full[:, :],
                        idxs_ap=gidx_t[:, b * IW:(b + 1) * IW],
                        num_idxs=IE, num_idxs_reg=IE, elem_size=H,
                        single_packet=False)
                    st = spool.tile([128, TB * 128], F32, tag="st")
                    nc.sync.dma_start(st[:, :],
                                      stiled[:, b * IE:(b + 1) * IE])
                    for kt in range(KH):
                        yps = ps2.tile([128, 128], F32, tag="yps")
                        for t in range(TB):
                            nc.tensor.matmul(
                                yps[:, :],
                                g1[:, t * H + kt * 128: t * H + (kt + 1) * 128],
                                st[:, t * 128:(t + 1) * 128],
                                start=(t == 0), stop=(t == TB - 1))
                        nc.vector.tensor_copy(
                            y1T[:, (kt * NB + b) * 128:(kt * NB + b + 1) * 128],
                            yps[:, :])

            # ---- phase 3a: x2 = y1 @ W2, AllGather ----------------------
            x2_bounce = dram.tile([R, O], F32)
            x2_full = dram.tile([N, O], F32, addr_space=shared)
            with tc.tile_pool(name="p3", bufs=1) as pool, \
                 tc.tile_pool(name="p3ps", bufs=2, space="PSUM") as ps3:
                x2all = pool.tile([128, NB * O], F32)
                for nt in range(NB):
                    x2ps = ps3.tile([128, O], F32, tag="x2ps")
                    for kt in range(KH):
                        nc.tensor.matmul(
                            x2ps[:, :],
                            y1T[:, (kt * NB + nt) * 128:(kt * NB + nt + 1) * 128],
                            w2_t[:, kt * O:(kt + 1) * O],
                            start=(kt == 0), stop=(kt == KH - 1))
                    nc.vector.tensor_copy(x2all[:, nt * O:(nt + 1) * O], x2ps[:, :])
                nfull = R // 128
                nc.sync.dma_start(
                    x2_bounce[:nfull * 128, :].rearrange("(b p) o -> p b o", p=128),
                    x2all[:, :nfull * O].rearrange("p (b o) -> p b o", o=O))
                if R % 128:
                    nc.sync.dma_start(x2_bounce[nfull * 128:, :],
                                      x2all[:R % 128, nfull * O:(nfull + 1) * O])

            nc.gpsimd.collective_compute(
                "AllGather", mybir.AluOpType.bypass, replica_groups=rg,
                ins=[x2_bounce.opt()], outs=[x2_full.opt()])
            y1cm.__exit__(None, None, None)

            # ---- phase 3b: zT = (A @ x2).T via G2.T @ S -----------------
            ztcm = tc.tile_pool(name="ztp", bufs=1)
            ztpool = ztcm.__enter__()
            zTsh = ztpool.tile([O, RP], F32)
            # decoder operands: bf16 hi/lo split of z (hi*hi + hi*lo + lo*hi
            # in fp32 PSUM accumulation recovers ~fp32 precision at bf16
            # matmul throughput; the dropped lo*lo term is ~2^-16 relative)
            BF16 = mybir.dt.bfloat16
            zshH = ztpool.tile([O, RP], BF16)
            zshL = ztpool.tile([O, RP], BF16)
            zalH = ztpool.tile([O, N], BF16)
            zalL = ztpool.tile([O, N], BF16)

            zT_bounce = dram.tile([O, R], F32)
            zT_full = dram.tile([NC, O, R], F32, addr_space=shared)
            with tc.tile_pool(name="p3b", bufs=2) as gpool, \
                 tc.tile_pool(name="p3bs", bufs=2) as spool, \
                 tc.tile_pool(name="p3bps", bufs=4, space="PSUM") as psz:
                for b in range(NB):
                    g2 = gpool.tile([128, TB * O], F32, tag="g2")
                    nc.gpsimd.dma_gather(
                        out_ap=g2[:, :].rearrange("p (t h) -> p t h", h=O),
                        in_ap=x2_full[:, :],
                        idxs_ap=gidx_t[:, b * IW:(b + 1) * IW],
                        num_idxs=IE, num_idxs_reg=IE, elem_size=O,
                        single_packet=False)
                    st = spool.tile([128, TB * 128], F32, tag="st2")
                    nc.sync.dma_start(st[:, :],
                                      stiled[:, b * IE:(b + 1) * IE])
                    zps = psz.tile([O, 128], F32, tag="zps")
                    for t in range(TB):
                        nc.tensor.matmul(
                            zps[:, :], g2[:, t * O:(t + 1) * O],
                            st[:, t * 128:(t + 1) * 128],
                            start=(t == 0), stop=(t == TB - 1))
                    nc.vector.tensor_copy(zTsh[:, b * 128:(b + 1) * 128], zps[:, :])
                    v = min(128, R - b * 128)
                    nc.sync.dma_start(zT_bounce[:, b * 128:b * 128 + v],
                                      zTsh[:, b * 128:b * 128 + v])

            nc.sync.dma_start(out_z[:, :], zTsh[:, :R])
            nc.gpsimd.collective_compute(
                "AllGather", mybir.AluOpType.bypass, replica_groups=rg,
                ins=[zT_bounce.opt()], outs=[zT_full.opt()])

            # hi/lo split of the local shard
            with tc.tile_pool(name="psp", bufs=2) as pool:
                tmp = pool.tile([O, RP], F32, tag="tmp0")
                nc.vector.tensor_copy(zshH[:, :], zTsh[:, :])      # cast dn
                nc.vector.tensor_copy(tmp[:, :], zshH[:, :])       # cast up
                nc.vector.tensor_tensor(tmp[:, :], zTsh[:, :], tmp[:, :],
                                        mybir.AluOpType.subtract)
                nc.vector.tensor_copy(zshL[:, :], tmp[:, :])
                # hi/lo split of the gathered z, chunked by rank block
                for r in range(NC):
                    zf = pool.tile([O, R], F32, tag="zf")
                    nc.sync.dma_start(zf[:, :], zT_full[r, :, :])
                    sl = slice(r * R, (r + 1) * R)
                    nc.vector.tensor_copy(zalH[:, sl], zf[:, :])
                    t2 = pool.tile([O, R], F32, tag="t2")
                    nc.vector.tensor_copy(t2[:, :], zalH[:, sl])
                    nc.vector.tensor_tensor(t2[:, :], zf[:, :], t2[:, :],
                                            mybir.AluOpType.subtract)
                    nc.vector.tensor_copy(zalL[:, sl], t2[:, :])

            # ---- phase 4: A_rec = sigmoid(z_shard @ z_full.T) -----------
            NCT = N // CW                  # decoder col tiles (within ranks)
            GRP = 4                        # col tiles per staging buffer
            with tc.tile_pool(name="p4o", bufs=3) as opool, \
                 tc.tile_pool(name="p4ps", bufs=8, space="PSUM") as ps4:
                for rt in range(NB):
                    v = min(128, R - rt * 128)
                    rsl = slice(rt * 128, (rt + 1) * 128)
                    for gc in range(0, NCT, GRP):
                        nct = min(GRP, NCT - gc)
                        ab = opool.tile([128, GRP * CW], F32, tag="ab")
                        for j in range(nct):
                            ct = gc + j
                            csl = slice(ct * CW, (ct + 1) * CW)
                            aps = ps4.tile([128, CW], F32, tag="aps")
                            nc.tensor.matmul(aps[:, :], zshH[:, rsl],
                                             zalH[:, csl], start=True, stop=False)
                            nc.tensor.matmul(aps[:, :], zshH[:, rsl],
                                             zalL[:, csl], start=False, stop=False)
                            nc.tensor.matmul(aps[:, :], zshL[:, rsl],
                                             zalH[:, csl], start=False, stop=True)
                            nc.scalar.activation(
                                ab[:, j * CW:(j + 1) * CW], aps[:, :],
                                mybir.ActivationFunctionType.Sigmoid)
                        nc.sync.dma_start(
                            out_a[rt * 128:rt * 128 + v,
                                  gc * CW:(gc + nct) * CW],
                            ab[:v, :nct * CW])
            ztcm.__exit__(None, None, None)

    nc.compile()
    return nc


_CACHE = {}


def kernel(feat, adj_rows, adj_cols, adj_vals, W1, W2):
    cfg = Cfg()
    in_maps, cfg = preprocess(feat, adj_rows, adj_cols, adj_vals, W1, W2, cfg)
    key = ("k", cfg.N, cfg.NC, cfg.TB)
    if key not in _CACHE:
        _CACHE[key] = build_bass(cfg)
    nc = _CACHE[key]
    res = bass_utils.run_bass_kernel_spmd(
        nc, in_maps, core_ids=list(range(cfg.NC)))
    z = np.concatenate([res.results[c]["out_z"].T for c in range(cfg.NC)], axis=0)
    A = np.concatenate([res.results[c]["out_a"] for c in range(cfg.NC)], axis=0)
    return z, A


# revision 23
# speedup vs baseline: 1.3213x; 1.1567x over previous
"""GNN encoder (2-layer GCN + inner-product decoder) on 8 TRN2 NeuronCores.

Sharding: nodes (rows of feat/z) are sharded across the 8 cores.
  phase 1:  x1 = feat @ W1          row-sharded GEMM (featT pre-transposed on
                                    host), AllGather x1
  phase 2:  y1 = A @ x1             per-core rows; dma_gather x1[cols]; the
                                    segment-sum is matmuls against host-built
                                    one-hot scatter matrices S (vals folded
                                    in); computed as y1T = G.T @ S
  phase 3a: x2 = y1 @ W2            from y1T, AllGather x2
  phase 3b: z = A @ x2              same spmm scheme, produces zT directly;
                                    AllGather zT (rank-blocked)
  phase 4:  A_rec = sigmoid(z_shard @ z_full.T)  row-sharded, K=64

All matmuls in fp32: the decoder logits have std ~2300 and the near-threshold
sigmoid entries are precision-critical; bf16/tf32 anywhere pushes the output
rel-err to ~1e-2.
"""

import math
import numpy as np

import concourse.bass as bass
import concourse.bacc as bacc
import concourse.mybir as mybir
import concourse.tile as tile
from concourse import bass_utils

F32 = mybir.dt.float32
I16 = mybir.dt.int16


class Cfg:
    def __init__(self, N=20000, NC=8, D=512, H=256, O=64, TB=None):
        self.N = N          # total nodes
        self.NC = NC        # cores
        self.D = D          # input feature dim
        self.H = H          # hidden dim
        self.O = O          # output dim
        assert N % NC == 0
        self.R = N // NC    # rows per core
        self.NB = (self.R + 127) // 128   # row blocks per core
        self.RP = self.NB * 128           # padded rows per core
        self.KD = D // 128  # K tiles for GEMM1
        self.KH = H // 128  # K tiles for GEMM2
        # decoder col tile: a divisor of R so tiles never cross rank blocks
        self.CW = next(cw for cw in (512, 500, 256, 128, 64)
                       if self.R % cw == 0)
        self.TB = TB        # edge tiles per row block (set by preprocess)


def preprocess(feat, adj_rows, adj_cols, adj_vals, W1, W2, cfg):
    """Host side: shard edges by destination row, group into row blocks of
    128, pad each (core, block) to a uniform TB tiles of 128 edges, and
    build the gather-index array plus the dense one-hot scatter matrices
    S[tile] (128 edges x 128 rows, entry = edge weight)."""
    N, NC, R, NB = cfg.N, cfg.NC, cfg.R, cfg.NB
    rows = np.asarray(adj_rows).astype(np.int64)
    cols = np.asarray(adj_cols).astype(np.int64)
    vals = np.asarray(adj_vals).astype(np.float32)
    feat = np.asarray(feat, dtype=np.float32)
    W1 = np.asarray(W1, dtype=np.float32)
    W2 = np.asarray(W2, dtype=np.float32)

    core_of = rows // R
    er = rows - core_of * R           # row local to core
    blk = er // 128                   # row block within core
    lr = er - blk * 128               # row local to block

    order = np.lexsort((blk, core_of))
    core_s, blk_s, lr_s, col_s, val_s = (
        core_of[order], blk[order], lr[order], cols[order], vals[order])

    counts = np.zeros((NC, NB), dtype=np.int64)
    np.add.at(counts, (core_s, blk_s), 1)
    TB = max(int(math.ceil(counts.max() / 128.0)), 1)
    cfg.TB = TB
    ntile_edges = TB * 128

    seg_end = np.cumsum(counts.reshape(-1))
    seg_start = np.concatenate([[0], seg_end[:-1]])

    in_maps = []
    for c in range(NC):
        gidx = np.zeros((NB, ntile_edges), dtype=np.int16)
        # S tiles: [NB*TB, 128 edges, 128 rows]
        stile = np.zeros((NB * TB, 128, 128), dtype=np.float32)
        for b in range(NB):
            s, e = seg_start[c * NB + b], seg_end[c * NB + b]
            n = e - s
            gidx[b, :n] = col_s[s:e].astype(np.int16)
            ti = b * TB + np.arange(n) // 128
            ei = np.arange(n) % 128
            stile[ti, ei, lr_s[s:e]] = val_s[s:e]
        gw = gidx.reshape(NB, ntile_edges // 16, 16).transpose(2, 0, 1)
        gw = gw.reshape(16, NB * (ntile_edges // 16))
        gidx_dev = np.tile(gw, (8, 1))  # idx i -> [i%16, i//16], 8x replicated
        # device layout: [128 edge-partitions, NB*TB*128 row-cols]
        stile_dev = stile.transpose(1, 0, 2).reshape(128, NB * TB * 128).copy()

        featT = np.zeros((cfg.D, cfg.RP), dtype=np.float32)
        featT[:, :R] = feat[c * R:(c + 1) * R].T

        in_maps.append({
            "featT": featT,
            "w1": W1,
            "w2": W2,
            "gidx": gidx_dev,
            "stile": stile_dev,
        })
    return in_maps, cfg


def build_bass(cfg, enable_asserts=False):
    N, NC, D, H, O = cfg.N, cfg.NC, cfg.D, cfg.H, cfg.O
    R, NB, RP, TB, CW = cfg.R, cfg.NB, cfg.RP, cfg.TB, cfg.CW
    KD, KH = cfg.KD, cfg.KH
    IE = TB * 128            # edges per block (padded)
    IW = IE // 16            # gidx columns per block

    nc = bacc.Bacc("TRN2", target_bir_lowering=False, debug=False,
                   enable_asserts=enable_asserts, num_devices=NC)

    featTd = nc.dram_tensor("featT", [D, RP], F32, kind="ExternalInput").ap()
    w1d = nc.dram_tensor("w1", [D, H], F32, kind="ExternalInput").ap()
    w2d = nc.dram_tensor("w2", [H, O], F32, kind="ExternalInput").ap()
    gidxd = nc.dram_tensor("gidx", [128, NB * IW], I16, kind="ExternalInput").ap()
    stiled = nc.dram_tensor("stile", [128, NB * TB * 128], F32,
                            kind="ExternalInput").ap()

    out_z = nc.dram_tensor("out_z", [O, R], F32, kind="ExternalOutput").ap()
    out_a = nc.dram_tensor("out_a", [R, N], F32, kind="ExternalOutput").ap()

    rg = [list(range(NC))]
    shared = "Shared" if NC > 4 else "Local"

    with tile.TileContext(nc) as tc:
        with tc.tile_pool(name="const", bufs=1) as cpool, \
             tc.tile_pool(name="dram", bufs=1, space="DRAM") as dram:

            # ---- constants / small inputs -------------------------------
            w1_t = cpool.tile([128, KD * H], F32)
            nc.sync.dma_start(
                w1_t[:, :].rearrange("p (t h) -> p t h", h=H),
                w1d.rearrange("(t p) h -> p t h", p=128))
            w2_t = cpool.tile([128, KH * O], F32)
            nc.sync.dma_start(
                w2_t[:, :].rearrange("p (t h) -> p t h", h=O),
                w2d.rearrange("(t p) h -> p t h", p=128))
            gidx_t = cpool.tile([128, NB * IW], I16)
            nc.sync.dma_start(gidx_t[:, :], gidxd)

            x1_bounce = dram.tile([R, H], F32)
            x1_full = dram.tile([N, H], F32, addr_space=shared)

            # ---- phase 1: x1 = feat @ W1, AllGather ---------------------
            with tc.tile_pool(name="p1", bufs=3) as pool, \
                 tc.tile_pool(name="p1ps", bufs=3, space="PSUM") as ps1:
                for nt in range(NB):
                    fT = pool.tile([128, KD * 128], F32, tag="fT")
                    nc.sync.dma_start(
                        fT[:, :].rearrange("p (t n) -> p t n", n=128),
                        featTd[:, nt * 128:(nt + 1) * 128]
                        .rearrange("(t p) n -> p t n", p=128))
                    x1ps = ps1.tile([128, H], F32, tag="x1ps")
                    for kt in range(KD):
                        nc.tensor.matmul(x1ps[:, :], fT[:, kt * 128:(kt + 1) * 128],
                                         w1_t[:, kt * H:(kt + 1) * H],
                                         start=(kt == 0), stop=(kt == KD - 1))
                    x1sb = pool.tile([128, H], F32, tag="x1sb")
                    nc.vector.tensor_copy(x1sb[:, :], x1ps[:, :])
                    v = min(128, R - nt * 128)
                    nc.sync.dma_start(x1_bounce[nt * 128:nt * 128 + v, :], x1sb[:v, :])

            nc.gpsimd.collective_compute(
                "AllGather", mybir.AluOpType.bypass, replica_groups=rg,
                ins=[x1_bounce.opt()], outs=[x1_full.opt()])

            # ---- phase 2: y1T = (A @ x1).T via G.T @ S ------------------
            y1cm = tc.tile_pool(name="y1p", bufs=1)
            y1pool = y1cm.__enter__()
            # y1T layout: [128 ch, (kt, b, 128 rows)] for kt in 0..KH-1
            y1T = y1pool.tile([128, KH * NB * 128], F32)
            with tc.tile_pool(name="p2g", bufs=3) as gpool, \
                 tc.tile_pool(name="p2s", bufs=2) as spool, \
                 tc.tile_pool(name="p2ps", bufs=4, space="PSUM") as ps2:
                for b in range(NB):
                    g1 = gpool.tile([128, TB * H], F32, tag="g1")
                    nc.gpsimd.dma_gather(
                        out_ap=g1[:, :].rearrange("p (t h) -> p t h", h=H),
                        in_ap=x1_full[:, :],
                        idxs_ap=gidx_t[:, b * IW:(b + 1) * IW],
                        num_idxs=IE, num_idxs_reg=IE, elem_size=H,
                        single_packet=False)
                    st = spool.tile([128, TB * 128], F32, tag="st")
                    nc.sync.dma_start(st[:, :],
                                      stiled[:, b * IE:(b + 1) * IE])
                    for kt in range(KH):
                        yps = ps2.tile([128, 128], F32, tag="yps")
                        for t in range(TB):
                            nc.tensor.matmul(
                                yps[:, :],
                                g1[:, t * H + kt * 128: t * H + (kt + 1) * 128],
                                st[:, t * 128:(t + 1) * 128],
                                start=(t == 0), stop=(t == TB - 1))
                        nc.vector.tensor_copy(
                            y1T[:, (kt * NB + b) * 128:(kt * NB + b + 1) * 128],
                            yps[:, :])

            # ---- phase 3a: x2 = y1 @ W2, AllGather ----------------------
            x2_bounce = dram.tile([R, O], F32)
            x2_full = dram.tile([N, O], F32, addr_space=shared)
            with tc.tile_pool(name="p3", bufs=1) as pool, \
                 tc.tile_pool(name="p3ps", bufs=2, space="PSUM") as ps3:
                x2all = pool.tile([128, NB * O], F32)
                for nt in range(NB):
                    x2ps = ps3.tile([128, O], F32, tag="x2ps")
                    for kt in range(KH):
                        nc.tensor.matmul(
                            x2ps[:, :],
                            y1T[:, (kt * NB + nt) * 128:(kt * NB + nt + 1) * 128],
                            w2_t[:, kt * O:(kt + 1) * O],
                            start=(kt == 0), stop=(kt == KH - 1))
                    nc.vector.tensor_copy(x2all[:, nt * O:(nt + 1) * O], x2ps[:, :])
                nfull = R // 128
                nc.sync.dma_start(
                    x2_bounce[:nfull * 128, :].rearrange("(b p) o -> p b o", p=128),
                    x2all[:, :nfull * O].rearrange("p (b o) -> p b o", o=O))
                if R % 128:
                    nc.sync.dma_start(x2_bounce[nfull * 128:, :],
                                      x2all[:R % 128, nfull * O:(nfull + 1) * O])

            nc.gpsimd.collective_compute(
                "AllGather", mybir.AluOpType.bypass, replica_groups=rg,
                ins=[x2_bounce.opt()], outs=[x2_full.opt()])
            y1cm.__exit__(None, None, None)

            # ---- phase 3b: zT = (A @ x2).T via G2.T @ S -----------------
            ztcm = tc.tile_pool(name="ztp", bufs=1)
            ztpool = ztcm.__enter__()
            zTsh = ztpool.tile([O, RP], F32)
            # decoder operands: bf16 hi/lo split of z (hi*hi + hi*lo + lo*hi
            # in fp32 PSUM accumulation recovers ~fp32 precision at bf16
            # matmul throughput; the dropped lo*lo term is ~2^-16 relative)
            # K=128-packed operands: lhsA = [hi|lo] stacked on partitions,
            # lhsB = [hi|0]; rhsHH = [zall_hi|zall_hi], rhsLL = [zall_lo|*].
            # A_rec psum = lhsA.T@rhsHH + lhsB.T@rhsLL
            #            = hi*hi + lo*hi + hi*lo  (fp32 accumulate)
            BF16 = mybir.dt.bfloat16
            lhsA = ztpool.tile([128, RP], BF16)
            lhsB = ztpool.tile([128, RP], BF16)
            rhsHH = ztpool.tile([128, N], BF16)
            rhsLL = ztpool.tile([128, N], BF16)

            zT_bounce = dram.tile([O, R], F32)
            zT_full = dram.tile([NC, O, R], F32, addr_space=shared)
            with tc.tile_pool(name="p3b", bufs=2) as gpool, \
                 tc.tile_pool(name="p3bs", bufs=2) as spool, \
                 tc.tile_pool(name="p3bps", bufs=4, space="PSUM") as psz:
                for b in range(NB):
                    g2 = gpool.tile([128, TB * O], F32, tag="g2")
                    nc.gpsimd.dma_gather(
                        out_ap=g2[:, :].rearrange("p (t h) -> p t h", h=O),
                        in_ap=x2_full[:, :],
                        idxs_ap=gidx_t[:, b * IW:(b + 1) * IW],
                        num_idxs=IE, num_idxs_reg=IE, elem_size=O,
                        single_packet=False)
                    st = spool.tile([128, TB * 128], F32, tag="st2")
                    nc.sync.dma_start(st[:, :],
                                      stiled[:, b * IE:(b + 1) * IE])
                    zps = psz.tile([O, 128], F32, tag="zps")
                    for t in range(TB):
                        nc.tensor.matmul(
                            zps[:, :], g2[:, t * O:(t + 1) * O],
                            st[:, t * 128:(t + 1) * 128],
                            start=(t == 0), stop=(t == TB - 1))
                    nc.vector.tensor_copy(zTsh[:, b * 128:(b + 1) * 128], zps[:, :])
                    v = min(128, R - b * 128)
                    nc.sync.dma_start(zT_bounce[:, b * 128:b * 128 + v],
                                      zTsh[:, b * 128:b * 128 + v])

            nc.sync.dma_start(out_z[:, :], zTsh[:, :R])
            nc.gpsimd.collective_compute(
                "AllGather", mybir.AluOpType.bypass, replica_groups=rg,
                ins=[zT_bounce.opt()], outs=[zT_full.opt()])

            # hi/lo split of the local shard
            with tc.tile_pool(name="psp", bufs=2) as pool:
                tmp = pool.tile([O, RP], F32, tag="tmp0")
                nc.vector.tensor_copy(lhsA[:O, :], zTsh[:, :])     # hi (cast)
                nc.vector.tensor_copy(tmp[:, :], lhsA[:O, :])      # cast up
                nc.vector.tensor_tensor(tmp[:, :], zTsh[:, :], tmp[:, :],
                                        mybir.AluOpType.subtract)
                nc.vector.tensor_copy(lhsA[O:, :], tmp[:, :])      # lo
                nc.vector.tensor_copy(lhsB[:O, :], lhsA[:O, :])
                nc.vector.memset(lhsB[O:, :], 0.0)
                # hi/lo split of the gathered z, chunked by rank block
                for r in range(NC):
                    zf = pool.tile([O, R], F32, tag="zf")
                    nc.sync.dma_start(zf[:, :], zT_full[r, :, :])
                    sl = slice(r * R, (r + 1) * R)
                    nc.vector.tensor_copy(rhsHH[:O, sl], zf[:, :])
                    nc.vector.tensor_copy(rhsHH[O:, sl], rhsHH[:O, sl])
                    t2 = pool.tile([O, R], F32, tag="t2")
                    nc.vector.tensor_copy(t2[:, :], rhsHH[:O, sl])
                    nc.vector.tensor_tensor(t2[:, :], zf[:, :], t2[:, :],
                                            mybir.AluOpType.subtract)
                    nc.vector.tensor_copy(rhsLL[:O, sl], t2[:, :])
                    nc.vector.tensor_copy(rhsLL[O:, sl], rhsLL[:O, sl])

            # ---- phase 4: A_rec = sigmoid(z_shard @ z_full.T) -----------
            NCT = N // CW                  # decoder col tiles (within ranks)
            GRP = 4                        # col tiles per staging buffer
            with tc.tile_pool(name="p4o", bufs=3) as opool, \
                 tc.tile_pool(name="p4ps", bufs=8, space="PSUM") as ps4:
                for rt in range(NB):
                    v = min(128, R - rt * 128)
                    rsl = slice(rt * 128, (rt + 1) * 128)
                    for gc in range(0, NCT, GRP):
                        nct = min(GRP, NCT - gc)
                        ab = opool.tile([128, GRP * CW], F32, tag="ab")
                        for j in range(nct):
                            ct = gc + j
                            csl = slice(ct * CW, (ct + 1) * CW)
                            aps = ps4.tile([128, CW], F32, tag="aps")
                            nc.tensor.matmul(aps[:, :], lhsA[:, rsl],
                                             rhsHH[:, csl], start=True, stop=False)
                            nc.tensor.matmul(aps[:, :], lhsB[:, rsl],
                                             rhsLL[:, csl], start=False, stop=True)
                            nc.scalar.activation(
                                ab[:, j * CW:(j + 1) * CW], aps[:, :],
                                mybir.ActivationFunctionType.Sigmoid)
                        nc.sync.dma_start(
                            out_a[rt * 128:rt * 128 + v,
                                  gc * CW:(gc + nct) * CW],
                            ab[:v, :nct * CW])
            ztcm.__exit__(None, None, None)

    nc.compile()
    return nc


_CACHE = {}


def kernel(feat, adj_rows, adj_cols, adj_vals, W1, W2):
    cfg = Cfg()
    in_maps, cfg = preprocess(feat, adj_rows, adj_cols, adj_vals, W1, W2, cfg)
    key = ("k", cfg.N, cfg.NC, cfg.TB)
    if key not in _CACHE:
        _CACHE[key] = build_bass(cfg)
    nc = _CACHE[key]
    res = bass_utils.run_bass_kernel_spmd(
        nc, in_maps, core_ids=list(range(cfg.NC)))
    z = np.concatenate([res.results[c]["out_z"].T for c in range(cfg.NC)], axis=0)
    A = np.concatenate([res.results[c]["out_a"] for c in range(cfg.NC)], axis=0)
    return z, A
